# revision 42
# baseline (speedup 1.0000x reference)
"""Criss-cross attention block (CCNet) Bass/Tile kernel for Trainium2.

Shapes (hardcoded): B=8, C=256, H=W=128, CR=32. Data-parallel over batch:
core b processes image b. Full inputs in, full output out.

Step1 vs baseline: x uploaded bf16 (host cast), output stored bf16,
residual + bz added on host. Otherwise identical to baseline.
"""
import sys

sys.path.insert(0, "/opt/trn_rl_repo")

import numpy as np
import ml_dtypes

import concourse.bass as bass
import concourse.mybir as mybir
from concourse import bacc, tile
from concourse.bass_utils import run_bass_kernel_spmd

B, C, H, W, CR = 8, 256, 128, 128, 32
HW = H * W
BF = ml_dtypes.bfloat16

_BUILD_CACHE = {}


def _build(with_qkv_bias: bool):
    nc = bacc.Bacc("TRN2", target_bir_lowering=False, debug=False, num_devices=8)
    dt = mybir.dt
    f32, bf16 = dt.float32, dt.bfloat16

    x_d = nc.dram_tensor("x", [C, HW], bf16, kind="ExternalInput").ap()
    wkqvT_d = nc.dram_tensor("wkqvT", [C, 96], bf16, kind="ExternalInput").ap()
    wzT_d = nc.dram_tensor("wzT", [CR, C], bf16, kind="ExternalInput").ap()
    mask_d = nc.dram_tensor("mask8", [128, 8, 128], bf16, kind="ExternalInput").ap()
    ident_d = nc.dram_tensor("identpad", [128, 32], bf16, kind="ExternalInput").ap()
    if with_qkv_bias:
        bvkq_d = nc.dram_tensor("bvkq", [1, 96], bf16, kind="ExternalInput").ap()

    zscr = nc.dram_tensor("zscr", [HW], bf16, kind="Internal").ap()
    rscr = nc.dram_tensor("rscr", [HW], bf16, kind="Internal").ap()
    out_d = nc.dram_tensor("out", [C, HW], bf16, kind="ExternalOutput").ap()

    with tile.TileContext(nc) as tc:
        with (
            tc.tile_pool(name="persist", bufs=1) as pp,
            tc.tile_pool(name="xroll", bufs=3) as xp,
            tc.tile_pool(name="work", bufs=2) as wp,
            tc.tile_pool(name="outw", bufs=6) as op,
            tc.tile_pool(name="rwork", bufs=5) as rp,
            tc.tile_pool(name="psA", bufs=2, space="PSUM") as pA,
            tc.tile_pool(name="psB", bufs=2, space="PSUM") as pB,
            tc.tile_pool(name="psC", bufs=2, space="PSUM") as pC,
        ):
            # ---- persistent SBUF ----
            # tA rows: K@0, Q@32, V@64 (row-major). tB rows: Q@0, K@32.
            # tC/tD: same layouts, col-major (w outer, h inner).
            tA = pp.tile([96, H, W], bf16)
            tB = pp.tile([64, H, W], bf16)
            tC = pp.tile([96, W, H], bf16)
            tD = pp.tile([64, W, H], bf16)
            out_u = pp.tile([33, H, W], bf16)  # attn out rows 0-31, Z row 32
            vts = pp.tile([128, W, 33], bf16)  # V^T stripes (+ones col)
            wkqvT = pp.tile([128, 2, 96], bf16)
            wzT = pp.tile([CR, C], bf16)
            mask8 = pp.tile([128, 8, 128], bf16)
            ident = pp.tile([128, 32], bf16)

            nc.sync.dma_start(out=wkqvT[:], in_=wkqvT_d.rearrange("(a p) m -> p a m", p=128))
            nc.sync.dma_start(out=wzT[:], in_=wzT_d)
            nc.sync.dma_start(out=mask8[:], in_=mask_d)
            nc.sync.dma_start(out=ident[:], in_=ident_d)
            if with_qkv_bias:
                ones_row = pp.tile([1, 512], bf16)
                nc.vector.memset(ones_row[:], 1.0)
                bvkq = pp.tile([1, 96], bf16)
                nc.sync.dma_start(out=bvkq[:], in_=bvkq_d)

            nc.vector.memset(vts[:, :, 32:33], 1.0)

            # ---- PE warmup: flip the HAM clock gate before real work ----
            warm_rhs = mask8[:, 0:4, :].rearrange("p a b -> p (a b)")
            for _ in range(10):
                pw = pA.tile([96, 512], f32, tag="pse")
                nc.tensor.matmul(pw[:], wkqvT[:, 0, :], warm_rhs, start=True, stop=True)

            def transpose_batch(b8, row_mode):
                """V^T stripes for 8 rows/cols starting at 8*b8 -> vts."""
                s0 = b8 * 8
                src_t = tA[64:96] if row_mode else tC[64:96]
                pv = pC.tile([128, 8, 32], bf16, tag="psc")
                for j in range(8):
                    nc.tensor.transpose(pv[:, j, :], src_t[:, s0 + j, :], ident[64:96, :])
                nc.vector.tensor_copy(vts[:, s0:s0 + 8, 0:32], pv[:])

            def attn_batch(b8, row_mode, expe_box):
                """Energies+exp for batch b8 (8 stripes)."""
                s0 = b8 * 8
                ps_e = pA.tile([128, 8, 128], f32, tag="pse")
                for j in range(8):
                    # 2-group row tiling: even stripes use PE rows 0-31, odd
                    # rows 32-63 (concurrent row tiles). Row tiles must write
                    # distinct PSUM banks: evens -> slots 0-3, odds -> 4-7.
                    if row_mode:
                        ksrc, qsrc = (tA[0:32], tB[0:32]) if j % 2 == 0 else (tB[32:64], tA[32:64])
                    else:
                        ksrc, qsrc = (tC[0:32], tD[0:32]) if j % 2 == 0 else (tD[32:64], tC[32:64])
                    lhsT, rhs = ksrc[:, s0 + j, :], qsrc[:, s0 + j, :]
                    slot = (j % 2) * 4 + j // 2
                    nc.tensor.matmul(ps_e[:, slot, :], lhsT, rhs, start=True, stop=True)
                expe = wp.tile([128, 8, 128], bf16, tag="expe")
                nc.scalar.activation(expe[:], ps_e[:], mybir.ActivationFunctionType.Exp)
                if not row_mode:
                    meng = nc.gpsimd if b8 % 2 == 0 else nc.vector
                    meng.tensor_mul(expe[:], expe[:], mask8[:])
                expe_box[b8] = expe

            def apply_batch(b8, row_mode, expe_box):
                """V^T @ exp for batch b8. Column tiling: even stripes ->
                psum partitions 0-32 (col tile 0), odd -> 64-96 (col tile 1),
                so two applies run concurrently in the PE array."""
                s0 = b8 * 8
                expe = expe_box[b8]
                for half in range(2):
                    ps_a = pC.tile([128, 4, 128], f32, tag="psc")
                    for jj in range(4):
                        j = half * 4 + jj
                        slot = (j % 2) * 4 + j // 2
                        p0 = (jj // 2) * 64
                        nc.tensor.matmul(ps_a[p0:p0 + 33, jj, :], vts[:, s0 + j, :],
                                         expe[:, slot, :], start=True, stop=True)
                    c0 = s0 + half * 4
                    for par in range(2):
                        # par=0: stripes c0,c0+1 at psum 0-32 (col tile 0);
                        # par=1: stripes c0+2,c0+3 at psum 64-96 (col tile 1)
                        src = ps_a[par * 64:par * 64 + 33, par * 2:par * 2 + 2, :]
                        cp = c0 + par * 2
                        if row_mode:
                            dst = out_u[:, cp:cp + 2, :]
                            nc.vector.tensor_copy(dst, src)
                        else:
                            # order-B APs: keep dst inner runs contiguous
                            dst = out_u[:, :, cp:cp + 2]
                            nc.vector.tensor_add(
                                dst, dst, src.rearrange("p w h -> p h w"))
                expe_box[b8] = None

            # ========== P1 + row attention, interleaved by eighths ==========
            expe_box = [None] * 16
            prev_rb = None
            for e in range(8):
                xts = []
                for h16 in range(2):
                    s16 = e * 2048 + h16 * 1024
                    xt = xp.tile([128, 2, 1024], bf16, tag="x")
                    nc.sync.dma_start(out=xt[:, 0, :], in_=x_d[0:128, s16:s16 + 1024])
                    nc.sync.dma_start(out=xt[:, 1, :], in_=x_d[128:256, s16:s16 + 1024])
                    xts.append(xt)
                for chl in range(4):
                    xt = xts[chl // 2]
                    s = (chl % 2) * 512
                    ps = pB.tile([96, 512], f32, tag="psb")
                    nc.tensor.matmul(ps[:], wkqvT[:, 0, :], xt[:, 0, s:s + 512],
                                     start=True, stop=False)
                    nc.tensor.matmul(ps[:], wkqvT[:, 1, :], xt[:, 1, s:s + 512],
                                     start=False, stop=not with_qkv_bias)
                    if with_qkv_bias:
                        nc.tensor.matmul(ps[:], bvkq[:], ones_row[:],
                                         start=False, stop=True)
                    h0 = e * 16 + chl * 4
                    ps3 = ps[:].rearrange("p (a b) -> p a b", b=128)
                    # row-major evacuation: one [96,512] copy on ACT
                    nc.scalar.copy(tA[0:96, h0:h0 + 4, :], ps3)
                    # col-major corner-turn from tA (SBUF->SBUF), order-B APs:
                    # dst inner runs contiguous, src reads strided
                    eng = nc.vector if chl % 2 == 0 else nc.gpsimd
                    eng.tensor_copy(
                        tC[0:96, :, h0:h0 + 4],
                        tA[0:96, h0:h0 + 4, :].rearrange("p h w -> p w h"))
                # replicate this eighth's Q/K to the alternate base (off-engine)
                r0 = e * 16
                nc.sync.dma_start(out=tB[0:32, r0:r0 + 16, :],
                                  in_=tA[32:64, r0:r0 + 16, :])   # Q@0
                nc.sync.dma_start(out=tB[32:64, r0:r0 + 16, :],
                                  in_=tA[0:32, r0:r0 + 16, :])    # K@32
                # row attention lags one eighth behind P1 so its dependency
                # chain (tB DMA -> energies -> exp) never stalls the in-order
                # PE queue: by the time the PE reaches these matmuls, their
                # inputs have been ready for a full eighth.
                for bl in range(2):
                    b8 = e * 2 + bl - 2
                    if b8 < 0:
                        continue
                    transpose_batch(b8, True)
                    attn_batch(b8, True, expe_box)
                    if prev_rb is not None:
                        apply_batch(prev_rb, True, expe_box)
                    prev_rb = b8
            for b8 in (14, 15):
                transpose_batch(b8, True)
                attn_batch(b8, True, expe_box)
                apply_batch(prev_rb, True, expe_box)
                prev_rb = b8
            apply_batch(prev_rb, True, expe_box)

            # col-major replicas Qc@0 / Kc@32 (fire as tC evacs complete)
            for wq in range(4):
                ws0 = wq * 32
                nc.sync.dma_start(out=tD[0:32, ws0:ws0 + 32, :],
                                  in_=tC[32:64, ws0:ws0 + 32, :])   # Qc@0
                nc.scalar.dma_start(out=tD[32:64, ws0:ws0 + 32, :],
                                    in_=tC[0:32, ws0:ws0 + 32, :])  # Kc@32
            # ========== column attention ==========
            prev = None
            for wb in range(17):
                if wb < 16:
                    transpose_batch(wb, False)
                    attn_batch(wb, False, expe_box)
                if prev is not None:
                    apply_batch(prev, False, expe_box)
                prev = wb if wb < 16 else None

            # ========== Z -> 1/Z ==========
            zsq = pp.tile([128, 128], bf16)
            nc.sync.dma_start(out=zsq[:],
                              in_=out_u[32:33, :, :].rearrange("p a b -> p (a b)"))
            rsq = pp.tile([128, 128], f32)
            nc.vector.reciprocal(rsq[:], zsq[:])
            rsq_bf = pp.tile([128, 128], bf16)
            nc.vector.tensor_copy(rsq_bf[:], rsq[:])
            nc.sync.dma_start(out=rscr.rearrange("(p f) -> p f", p=128), in_=rsq_bf[:])

            # ========== P5: normalize, Wz, store (residual+bz on host) ==========
            for cch in range(32):
                px = cch * 512
                h0 = cch * 4
                rb = rp.tile([32, 4, 128], bf16, tag="rb")
                src = rscr[px:px + 512].rearrange("(a b) -> a b", b=128)
                bcast = bass.AP(tensor=src.tensor, offset=src.offset,
                                ap=[[0, 32]] + list(src.ap))
                # rb broadcasts on the idle GPSIMD SWDGE queue, out stores on
                # sync -- separate FIFOs so stores don't stall the prefetches
                nc.gpsimd.dma_start(out=rb[:], in_=bcast)
                norm = rp.tile([32, 4, 128], bf16, tag="norm")
                nc.vector.tensor_mul(norm[:], out_u[0:32, h0:h0 + 4, :], rb[:])
                rhs = norm[:].rearrange("p a b -> p (a b)")
                # psum alternates between pool A (idle in P5) and pool B
                # for a deeper store pipeline
                if cch % 2 == 0:
                    psf2 = pA.tile([128, 2, 512], f32, tag="pse")
                else:
                    psf2 = None
                for half in range(2):
                    if psf2 is not None:
                        ps_f = psf2[:, half, :]
                    else:
                        psf1 = pB.tile([128, 512], f32, tag="psb")
                        ps_f = psf1[:]
                    nc.tensor.matmul(ps_f, wzT[:, half * 128:(half + 1) * 128],
                                     rhs, start=True, stop=True)
                    of = op.tile([128, 512], bf16, tag="of")
                    if half == 0:
                        if cch % 2 == 0:
                            nc.scalar.copy(of[:], ps_f)
                        else:
                            nc.vector.tensor_copy(of[:], ps_f)
                    else:
                        if cch % 2 == 0:
                            nc.vector.tensor_copy(of[:], ps_f)
                        else:
                            nc.scalar.copy(of[:], ps_f)
                    eng = nc.sync if half == 0 else nc.gpsimd
                    eng.dma_start(out=out_d[half * 128:(half + 1) * 128, px:px + 512],
                                  in_=of[:])
    nc.compile()
    return nc


def _host_prep(Wq, bq, Wk, bk, Wv, bv, Wz, bz):
    wkqvT = np.ascontiguousarray(
        np.concatenate([Wk, Wq, Wv], axis=0).T).astype(BF)          # (256, 96)
    wzT = np.ascontiguousarray(Wz.T).astype(BF)                      # (32, 256)
    eye = np.eye(128, dtype=np.float32)
    mask8 = np.ascontiguousarray(
        np.broadcast_to((1.0 - eye)[:, None, :], (128, 8, 128))).astype(BF)
    identpad = np.vstack([np.eye(32, dtype=np.float32)] * 4).astype(BF)
    bvkq = np.concatenate([bk, bq, bv]).reshape(1, 96).astype(BF)
    return wkqvT, wzT, mask8, identpad, bvkq


def _make_in_maps(x, Wq, bq, Wk, bk, Wv, bv, Wz, bz):
    wkqvT, wzT, mask8, identpad, bvkq = _host_prep(
        np.asarray(Wq, np.float32), np.asarray(bq, np.float32),
        np.asarray(Wk, np.float32), np.asarray(bk, np.float32),
        np.asarray(Wv, np.float32), np.asarray(bv, np.float32),
        np.asarray(Wz, np.float32), np.asarray(bz, np.float32))
    with_qkv_bias = bool(np.any(bvkq.astype(np.float32) != 0.0))
    x_bf = np.asarray(x, np.float32).reshape(B, C, HW).astype(BF)
    in_maps = []
    for b in range(B):
        m = dict(
            x=np.ascontiguousarray(x_bf[b]),
            wkqvT=wkqvT, wzT=wzT, mask8=mask8, identpad=identpad,
        )
        if with_qkv_bias:
            m["bvkq"] = bvkq
        in_maps.append(m)
    return in_maps, with_qkv_bias


def kernel(x, Wq, bq, Wk, bk, Wv, bv, Wz, bz):
    x = np.asarray(x, np.float32)
    in_maps, with_qkv_bias = _make_in_maps(x, Wq, bq, Wk, bk, Wv, bv, Wz, bz)

    if with_qkv_bias not in _BUILD_CACHE:
        _BUILD_CACHE[with_qkv_bias] = _build(with_qkv_bias)
    nc = _BUILD_CACHE[with_qkv_bias]

    res = run_bass_kernel_spmd(nc, in_maps, core_ids=list(range(8)))
    delta = np.stack([res.results[b]["out"].astype(np.float32).reshape(C, H, W)
                      for b in range(B)])
    out = delta + x.reshape(B, C, H, W)
    bz_f = np.asarray(bz, np.float32)
    if np.any(bz_f != 0.0):
        out = out + bz_f[None, :, None, None]
    return out
